# revision 1
# baseline (speedup 1.0000x reference)
"""CustomGaussianLayer Trainium2 kernel.

Math: out[b,o] = sum_{i,g} exp(-0.5*((tanh(x[b,i])-c_g)/w)^2) * coeff[o,i,g]*W[o,i]
 == E @ W2T  with  E[b, k=(g,i)] Gaussian basis,  W2T[k, o] folded weights.

Factored basis:  exp(-a*(t-c)^2) = exp(-a*t^2) * exp(2ac*t - a*c^2),  a = 24.5.
Per core (data-parallel over batch, 1024 rows each):
  ACT: tanh, exp ;  DVE: squares+mults ;  PE: f32r matmuls [o,b] += W2T_k^T E_k.
Host folds W2T = coeff*W, pre-transposes x; output gathered/transposed back.
"""

import numpy as np

import concourse.bacc as bacc
import concourse.bass as bass
import concourse.mybir as mybir
import concourse.tile as tile
from concourse.bass_utils import run_bass_kernel_spmd
from concourse.tile import add_dep_helper

G = 8
I_SZ = 512
O_SZ = 512
B = 8192
NCORES = 8
B_SH = B // NCORES          # 1024 batch rows per core
K = I_SZ * G                # 4096 contraction
N_IBLK = I_SZ // 128        # 4 partition blocks of i
FREE = N_IBLK * B_SH        # 4096 free layout (i_blk, b)
HALF = FREE // 2            # 2048 (i_blk 0-1 | 2-3)
N_OT = O_SZ // 128          # 4 output tiles
N_BC = B_SH // 512          # 2 batch chunks of 512 (psum free limit fp32)

ALPHA = 24.5
N_WARMUP = 8                # 0.5 / width^2, width = 1/7
CENTERS = np.linspace(-1.0, 1.0, G).astype(np.float32)

F32 = mybir.dt.float32
F32R = mybir.dt.float32r
AF = mybir.ActivationFunctionType
ALU = mybir.AluOpType

_NC_CACHE = {}


def build_nc():
    nc = bacc.Bacc("TRN2", target_bir_lowering=False)
    xt_d = nc.dram_tensor("xt", [I_SZ, B_SH], F32, kind="ExternalInput")
    w2t_d = nc.dram_tensor("w2t", [K, O_SZ], F32R, kind="ExternalInput")
    out_d = nc.dram_tensor("out_t", [O_SZ, B_SH], F32, kind="ExternalOutput")

    with tile.TileContext(nc) as tc:
        with (
            tc.tile_pool(name="w2", bufs=1) as w2_pool,
            tc.tile_pool(name="xt", bufs=1) as xt_pool,
            tc.tile_pool(name="tt", bufs=1) as tt_pool,
            tc.tile_pool(name="sq", bufs=1) as sq_pool,
            tc.tile_pool(name="aa", bufs=2) as aa_pool,
            tc.tile_pool(name="bb", bufs=3) as bb_pool,
            tc.tile_pool(name="ee", bufs=4) as ee_pool,
            tc.tile_pool(name="ps", bufs=1, space="PSUM") as ps_pool,
            tc.tile_pool(name="ob", bufs=1) as ob_pool,
        ):
            # trigger the ACT spline-table load immediately (costs ~1.3us;
            # otherwise it delays the first tanh)
            actwarm = tt_pool.tile([128, 1], F32, tag="actwarm")
            nc.scalar.activation(
                actwarm[:], nc.const_aps.tensor(0.0, (128, 1)), AF.Exp,
            )

            w2_all = w2_pool.tile([128, (K // 128) * O_SZ], F32R, tag="w2all")
            w2t_v = w2t_d[:, :].rearrange("(kt p) o -> p kt o", p=128)
            xt_sb = xt_pool.tile([128, FREE], F32, tag="xt")
            tt = tt_pool.tile([128, FREE], F32, tag="tt")
            xt_v = xt_d[:, :].rearrange("(ib p) b -> p ib b", p=128)

            def w2_dma(kt_lo, kt_hi):
                return nc.sync.dma_start(
                    w2_all[:, kt_lo * O_SZ:kt_hi * O_SZ]
                    .rearrange("p (kt o) -> p kt o", o=O_SZ),
                    w2t_v[:, kt_lo:kt_hi, :],
                )

            def xt_dma(ib_lo, ib_hi):
                return nc.sync.dma_start(
                    xt_sb[:, ib_lo * B_SH:ib_hi * B_SH]
                    .rearrange("p (ib b) -> p ib b", b=B_SH),
                    xt_v[:, ib_lo:ib_hi, :],
                )

            # SP DMA lane in consumer order; pin the order explicitly
            dma_chain = [
                xt_dma(0, 1),          # ib0        -> first basis quarter
                w2_dma(0, 1),          # kt0: feeds PE warm-up
                xt_dma(1, 2),          # ib1
                w2_dma(1, 4),          # kt 1-3  (h0 g0-1)
                xt_dma(2, 4),          # h1 x
                w2_dma(4, 8),          # kt 4-7  (h0 g2-3)
                w2_dma(8, 16),         # h0 g4-7
                w2_dma(16, 24),        # h1 g0-3
                w2_dma(24, 32),        # h1 g4-7
            ]
            for i in range(1, len(dma_chain)):
                add_dep_helper(
                    dma_chain[i].ins, dma_chain[i - 1].ins, sync=False,
                    reason="SP DMA lane consumer order",
                )

            # PE warm-up: dummy self-contained matmuls on the first w2 block,
            # overwritten later by the real start=True accumulation
            psum = [
                [
                    ps_pool.tile(
                        [128, 512], F32,
                        name=f"ps{ot}_{bc}", tag=f"ps{ot}_{bc}",
                    )
                    for bc in range(N_BC)
                ]
                for ot in range(N_OT)
            ]
            # gate only on the tiny first w2 DMA so they fill the startup
            # window; lhsT/rhs regions kept disjoint
            wu_lhs = w2_all[:, 0:128]
            wu_rhs = w2_all[:, 128:512]
            for w in range(N_WARMUP):
                nc.tensor.matmul(
                    psum[0][0][:, 0:384], wu_lhs, wu_rhs,
                    start=(w == 0), stop=(w == N_WARMUP - 1),
                )

            b_insts = []
            for h in range(2):
                sl = slice(h * HALF, (h + 1) * HALF)
                # first half runs at quarter granularity to shorten the
                # startup dependency chain; ACT order is pinned along the
                # critical path tanh_a -> B_0a -> A_a -> tanh_b -> ...
                parts = [(0, B_SH), (B_SH, HALF)] if h == 0 else [(0, HALF)]
                sq = sq_pool.tile([128, HALF], F32, tag="sq")
                a_t = aa_pool.tile([128, HALF], F32, tag="aa")
                tanh_is, a_is, sq_is = [], [], []
                c0 = float(CENTERS[0])
                b0_t = bb_pool.tile([128, HALF], F32, tag="bb")
                e0_t = ee_pool.tile([128, HALF], F32R, tag="ee")
                b0_is, e0_is = [], []
                for pi, (lo, hi) in enumerate(parts):
                    psl = slice(h * HALF + lo, h * HALF + hi)
                    t_i = nc.scalar.activation(tt[:, psl], xt_sb[:, psl], AF.Tanh)
                    tanh_is.append(t_i)
                    if pi > 0:
                        add_dep_helper(t_i.ins, a_is[pi - 1].ins, sync=False,
                                       reason="act chain")
                    sq_i = nc.vector.tensor_tensor(
                        sq[:, lo:hi], tt[:, psl], tt[:, psl], op=ALU.mult)
                    if pi > 0:
                        add_dep_helper(sq_i.ins, e0_is[pi - 1].ins, sync=False,
                                       reason="dve chain")
                    sq_is.append(sq_i)
                    b_i = nc.scalar.activation(
                        b0_t[:, lo:hi], tt[:, psl], AF.Exp,
                        scale=float(2.0 * ALPHA * c0),
                    )
                    b0_is.append(b_i)
                    a_i = nc.scalar.activation(
                        a_t[:, lo:hi], sq[:, lo:hi], AF.Exp, scale=-ALPHA)
                    add_dep_helper(a_i.ins, b_i.ins, sync=False,
                                   reason="act chain")
                    a_is.append(a_i)
                    e_i = nc.vector.tensor_tensor(
                        e0_t[:, lo:hi], a_t[:, lo:hi], b0_t[:, lo:hi],
                        op=ALU.mult)
                    e0_is.append(e_i)
                if h == 0:
                    b_insts.extend(b0_is)
                if h == 1:
                    # don't let the scheduler hoist h1 tanh ahead of the h0
                    # basis stream (head-of-line blocks ACT on its DMA)
                    add_dep_helper(
                        tanh_is[0].ins, b_insts[4].ins, sync=False,
                        reason="tanh_h1 after mid-h0 basis",
                    )

                for g in range(G):
                    c = float(CENTERS[g])
                    if g == 0:
                        b_t, e_t = b0_t, e0_t
                    else:
                        b_t = bb_pool.tile([128, HALF], F32, tag="bb")
                        e_t = ee_pool.tile([128, HALF], F32R, tag="ee")
                        b_i = nc.scalar.activation(
                            b_t[:], tt[:, sl], AF.Exp,
                            scale=float(2.0 * ALPHA * c),
                        )
                        if h == 0:
                            b_insts.append(b_i)
                        nc.vector.tensor_tensor(
                            e_t[:], a_t[:], b_t[:], op=ALU.mult)

                    for ib_loc in range(2):
                        kt = h * 16 + g * 2 + ib_loc
                        first = (h == 0) and (g == 0) and (ib_loc == 0)
                        last = (h == 1) and (g == G - 1) and (ib_loc == 1)
                        for ot in range(N_OT):
                            lhsT = w2_all[:, kt * O_SZ + ot * 128: kt * O_SZ + (ot + 1) * 128]
                            for bc in range(N_BC):
                                rhs = e_t[:, ib_loc * B_SH + bc * 512: ib_loc * B_SH + (bc + 1) * 512]
                                nc.tensor.matmul(
                                    psum[ot][bc][:], lhsT, rhs,
                                    start=first, stop=last,
                                )

            # --- drain psum -> SBUF (ACT+DVE split, ot-major) -> 4 DMAs out
            o_sb = ob_pool.tile([128, N_OT * N_BC * 512], F32, tag="osb")
            for ot in range(N_OT):
                for bc in range(N_BC):
                    dst = o_sb[:, (ot * N_BC + bc) * 512:(ot * N_BC + bc + 1) * 512]
                    if bc == 0:
                        nc.vector.tensor_copy(dst, psum[ot][bc][:])
                    else:
                        nc.scalar.activation(dst, psum[ot][bc][:], AF.Copy)
                out_eng = nc.sync if ot % 2 == 0 else nc.scalar
                out_eng.dma_start(
                    out_d[ot * 128:(ot + 1) * 128, :]
                    .rearrange("p (bc c) -> p bc c", c=512),
                    o_sb[:, ot * 1024:(ot + 1) * 1024]
                    .rearrange("p (bc c) -> p bc c", c=512),
                )
    nc.compile()
    return nc


def get_nc():
    if "nc" not in _NC_CACHE:
        _NC_CACHE["nc"] = build_nc()
    return _NC_CACHE["nc"]


def prep_inputs(x, weights, coefficients):
    x = np.asarray(x, dtype=np.float32)
    weights = np.asarray(weights, dtype=np.float32)
    coefficients = np.asarray(coefficients, dtype=np.float32)
    # W2T[k=g*I+i, o] = coeff[o,i,g] * W[o,i]
    w2t = (coefficients * weights[:, :, None]).transpose(2, 1, 0).reshape(K, O_SZ)
    # fold exp(-a*c_g^2) (from the factored Gaussian) into the weights
    gauss_bias = np.exp(-ALPHA * CENTERS.astype(np.float64) ** 2)  # [G]
    w2t = np.ascontiguousarray(
        (w2t.reshape(G, I_SZ, O_SZ) * gauss_bias[:, None, None]).reshape(K, O_SZ),
        dtype=np.float32,
    )
    # reorder k-tiles into device consumption order: pos = h*16 + g*2 + ib_loc
    # (source tile index = g*4 + 2h + ib_loc)
    w2t = w2t.reshape(G, N_IBLK, 128, O_SZ)           # [g, ib, p, o]
    w2t = w2t.transpose(1, 0, 2, 3).reshape(2, 2, G, 128 * O_SZ)  # [h, ib_loc->? ]
    # careful: transpose(1,0,..) gives [ib, g, ...]; ib = 2h + ib_loc ->
    # axis order [ib(4), g, ...] -> view as [h(2), ib_loc(2), g, ...]
    w2t = w2t.reshape(2, 2, G, 128 * O_SZ).transpose(0, 2, 1, 3)  # [h, g, ib_loc, ...]
    w2t = np.ascontiguousarray(w2t.reshape(K, O_SZ))
    # round-to-nearest-even to fp32r (8-bit exp, 11-bit mantissa; low 12 bits 0)
    u = w2t.view(np.uint32)
    u[:] = (u + 0x7FF + ((u >> 12) & 1)) & 0xFFFFF000
    xT = np.ascontiguousarray(x.T)  # [I, B]
    in_maps = [
        {
            "xt": np.ascontiguousarray(xT[:, c * B_SH:(c + 1) * B_SH]),
            "w2t": w2t,
        }
        for c in range(NCORES)
    ]
    return in_maps


def kernel(x, weights, coefficients):
    nc = get_nc()
    in_maps = prep_inputs(x, weights, coefficients)
    res = run_bass_kernel_spmd(nc, in_maps, core_ids=list(range(NCORES)))
    out = np.empty((B, O_SZ), dtype=np.float32)
    for c in range(NCORES):
        out[c * B_SH:(c + 1) * B_SH, :] = res.results[c]["out_t"].T
    return out

